# revision 44
# baseline (speedup 1.0000x reference)
"""NonLocal2D block (SAGAN-style non-local attention) on 8 Trainium2 cores.

Data-parallel over batch: core b computes batch element b entirely on-chip.

Math (per batch, N = 64*64 = 4096):
  f = Wf@x+bf [16,N], g = Wg@x+bg [16,N], h = Wh@x+bh [128,N]
  S = f^T g [N,N]; A = softmax_rows(S); att = h @ A; out = x + gamma*att

Design:
- The S matmul runs in fp8 DoubleRow perf mode (0.5 cyc/col) with a
  broadcast (stride-0) Ko=2 dim on both operands: every f.g product is
  summed twice, compensated by folding 1/sqrt(2) into each of the f and g
  scales (the kappa constant channels use a 0.5 g-side bias instead).
- exp work is split across TWO engines per strip, per an auto-tuned
  per-strip chunk template (_PAT): 'A' chunks run exact exp on ACT with
  free accum_out row sums; 'V' chunks run a Schraudolph bit-trick exp on
  DVE (one tensor_scalar clamp into int8 whose bits ARE the fp8e5 value).
  The Schraudolph affine  s1*S + c  is pre-folded into the S matmul:
  f/g carry two extra constant kappa channels providing c (48 + 0.8125).
  Per-row softmax shift/scale invariance makes scale conventions cancel.
- E is stored in fp8e5 PAIR tiles [128, 2, 4096]; the attended matmul
  runs in fp8 DoubleRow (2 strips per matmul at 0.5 cyc/col).
- 'V'-chunk row sums: one GPSIMD pairwise add over the E chunks, then a
  short DVE f16 reduce (deferred 2 strips so DVE never waits on GPS).
  ACT accum slots are combined on GPSIMD.
- Attended accumulates in two windows (pairs 0..W0_PAIRS-1 chain during
  strips 24..31; the rest is a short tail).  Each fold is ONE DVE
  scalar_tensor_tensor: att = pa*2^-K (+ x32 | + att).
- hts (= h^T * gamma*2^K/D) feeds the DoubleRow stationary in fp8e4.
- Engine assignment tables (chunk templates, evac/hts/fold engines,
  W0_PAIRS) were auto-tuned by coordinate descent against TimelineSim.
"""

import math

import numpy as np
import ml_dtypes

import concourse.bass as bass
import concourse.bacc as bacc
import concourse.tile as tile
import concourse.mybir as mybir
from concourse.bass_utils import run_bass_kernel_spmd

B, C, W, H = 8, 128, 64, 64
N = W * H          # 4096
P = 128
NSTRIP = N // P    # 32
NPAIR = NSTRIP // 2
MBLK = 512
NMB = N // MBLK    # 8
KCH = 18           # 16 f/g channels + 2 kappa constant channels
CHUNKS = [(0, 1024), (1024, 1024), (2048, 1024), (3072, 1024)]
W0_PAIRS = 12      # pairs 0..11 chain during strips 24..31; 12..15 in tail

# Schraudolph / scaling constants (see module docstring)
S1 = 4.0 / math.log(2.0)          # PSUM pre-scale: psum = s1*S + c
SQS1 = math.sqrt(S1)
CK1 = 48.0                        # coarse kappa channel (exact in fp8e4)
CK2 = 0.8125                      # fine kappa channel (exact in fp8e4)
B_SHIFT = 2.0                     # exp(S - b) convention for ACT strips
RATIO_LN = 0.013343      # ln(mean E_schraudolph / exp(S-b))
EXP_BIAS = -((CK1 + CK2) / S1 + B_SHIFT) + RATIO_LN
CLAMP_HI = 123.0                  # max finite fp8e5 bit pattern

F32 = mybir.dt.float32
BF16 = mybir.dt.bfloat16
F16 = mybir.dt.float16
E4 = mybir.dt.float8e4
E5 = mybir.dt.float8e5
I8 = mybir.dt.int8
EXP = mybir.ActivationFunctionType.Exp
IDENT = mybir.ActivationFunctionType.Identity
MUL = mybir.AluOpType.mult
ADD = mybir.AluOpType.add
MAX = mybir.AluOpType.max
MIN = mybir.AluOpType.min
DRMODE = mybir.MatmulPerfMode.DoubleRow

# strip -> exp engine: 'A' (ACT exact exp) or 'V' (DVE Schraudolph).
# per-strip chunk-engine pattern: chunks alternate ACT (exact exp) / DVE
# (Schraudolph).  Every ACT chunk carries accum_out row sums (accs slots);
# DVE chunks are row-summed by a GPS pairwise add + one DVE reduce.
# STRIPS3A strips run 3 ACT chunks to balance ACT vs DVE totals.
STRIPS3A = {3, 6, 9, 12, 15, 19, 22, 25, 27, 28, 30, 31}
# chunk layout per strip: W = cols 0:2048 (one wide ACT exp, accum slot 0),
# N1 = 2048:3072, N2 = 3072:4096.  N2 runs on ACT (accum slot 1) for
# STRIPS3A strips, else DVE.  N-chunks not accumulated go through the
# GPS add + DVE reduce rowsum path.
CHUNK_ENG = [('A', 'V', 'A', 'A') if s in STRIPS3A
             else ('A', 'V', 'A', 'V') for s in range(NSTRIP)]
CHUNK_ACC = [tuple(c == 'A' for c in p) for p in CHUNK_ENG]
ACC_SLOT = []
for p_ in CHUNK_ACC:
    slot, sl = 0, []
    for acc in p_:
        sl.append(slot if acc else -1)
        slot += int(acc)
    ACC_SLOT.append(tuple(sl))
# f/g PSUM->SBUF evacuation engine per projection block (16 total: f0..7,g0..7)
EVAC_F = ['A'] * 8
EVAC_G = ['A', 'A', 'A', 'V', 'V', 'V', 'V', 'V']
# ACT strips whose row sum runs on GPS (stt over E halves) instead of ACT
# accum_out; offloads the accum aux reads from the ACT critical path.
ROWSUM_GPS_A = set()
# per-strip hts-scale engine ('A' ACT copy / 'V' DVE tensor_scalar)
HTS_ENG = ['V' if s < 22 else 'A' for s in range(NSTRIP)]
# fold engine per block for window0 / window1 (x and att are folded into
# the matmul chain via an identity*2^K stationary, so the fold is a pure
# Copy+scale and can run on either engine)
FOLD_W0 = ['V'] * 8
FOLD_W1 = ['V'] * 8
# W0 chain length per block j (block j runs at strip 24+j; pairs complete
# through 11+floor((1+j)/2), pair 15 only after the loop)
W0J = [12, 12, 12, 12, 12, 12, 12, 12]
# engines whose DGE queues carry the 8 output DMAs (round-robin)
OUT_DMA_ENGS = ['sync']
F32R = mybir.dt.float32r
SMALL_BUFS = 6
SCRP_BUFS = 5

_NC = None


def _build():
    nc = bacc.Bacc(None, target_bir_lowering=False)
    x32 = nc.dram_tensor("x32", [P, N], F32R, kind="ExternalInput")
    xbf = nc.dram_tensor("xbf", [P, N], BF16, kind="ExternalInput")
    # wpack: [wft(18) | wgt(18) @32 | whT @64 | row0: bh @192] packed on host
    wpack = nc.dram_tensor("wpack", [P, 320], BF16, kind="ExternalInput")
    # fpack cols: 0 fbias | 1 gbias | 2 gam2k | 3 expbias | 4 inv2k
    fpack = nc.dram_tensor("fpack", [P, 5], F32, kind="ExternalInput")
    out = nc.dram_tensor("out", [P, N], F32, kind="ExternalOutput")

    with tile.TileContext(nc) as tc:
        with (
            tc.tile_pool(name="consts", bufs=1) as consts,
            tc.tile_pool(name="small", bufs=SMALL_BUFS) as small,
            tc.tile_pool(name="scrp", bufs=SCRP_BUFS) as scrp,
            tc.tile_pool(name="psS", bufs=3, space="PSUM") as psS,
            tc.tile_pool(name="psA", bufs=2, space="PSUM") as psA,
        ):
            wpack_s = consts.tile([P, 320], BF16)
            fpack_s = consts.tile([P, 5], F32)
            xbf_t = [consts.tile([P, MBLK], BF16, tag=f"xbf{j}", name=f"xbf{j}")
                     for j in range(NMB)]
            x32_t = [consts.tile([P, MBLK], F32R, tag=f"x32{j}",
                              name=f"x32{j}")
                     for j in range(NMB)]
            nc.sync.dma_start(wpack_s[:], wpack[:])
            nc.gpsimd.dma_start(xbf_t[0][:], xbf[:, 0:MBLK])
            nc.sync.dma_start(fpack_s[:], fpack[:])
            nc.gpsimd.dma_start(xbf_t[1][:], xbf[:, MBLK:2 * MBLK])
            nc.gpsimd.dma_start(xbf_t[2][:], xbf[:, 2 * MBLK:3 * MBLK])
            for j in range(3, NMB):
                nc.sync.dma_start(xbf_t[j][:], xbf[:, j * MBLK:(j + 1) * MBLK])
            for j in range(NMB):
                nc.sync.dma_start(x32_t[j][:], x32[:, j * MBLK:(j + 1) * MBLK])

            wft_s = wpack_s[:, 0:KCH]
            wgt_s = wpack_s[:, 32:32 + KCH]
            wht_s = wpack_s[:, 64:192]
            bhr_s = wpack_s[0:1, 192:320]
            fbias = fpack_s[:, 0:1]
            gbias = fpack_s[:, 1:2]
            gam2k = fpack_s[:, 2:3]
            ebias = fpack_s[:, 3:4]
            inv2k = fpack_s[:, 4:5]
            ones_s = consts.tile([1, P], BF16)
            nc.vector.memset(ones_s[:], 1.0)
            neg6_s = consts.tile([P, 1], F32)
            nc.vector.memset(neg6_s[:], -6.0)
            # dummy exp: pulls the ACT table load to t=0
            warm = small.tile([P, 1], F32, tag="warm")
            nc.scalar.activation(out=warm[:], in_=neg6_s[:], func=EXP)

            f_sb = [consts.tile([KCH, MBLK], E4, tag=f"fsb{j}", name=f"fsb{j}")
                    for j in range(NMB)]
            g_sb = [consts.tile([KCH, MBLK], E4, tag=f"gsb{j}", name=f"gsb{j}")
                    for j in range(NMB)]
            e_p = [consts.tile([P, 2, N], E5, tag=f"ep{p}", name=f"ep{p}")
                   for p in range(NPAIR)]
            hts_p = [consts.tile([P, 2, P], E4, tag=f"hp{p}", name=f"hp{p}")
                     for p in range(NPAIR)]
            att_t = [consts.tile([P, MBLK], F32, tag=f"att{j}", name=f"att{j}")
                     for j in range(NMB)]

            def fg_block(j, which):
                """Project f/g block j and evacuate to fp8e4 SBUF with bias."""
                ps = psA.tile([P, MBLK], F32, tag="att", name=f"ps{which}{j}")
                wsl = wft_s if which == "f" else wgt_s
                dst = (f_sb if which == "f" else g_sb)[j]
                bias = fbias if which == "f" else gbias
                ev = (EVAC_F if which == "f" else EVAC_G)[j]
                nc.tensor.matmul(ps[0:KCH, :], wsl, xbf_t[j][:],
                                 start=True, stop=True)
                if ev == 'A':
                    nc.scalar.activation(out=dst[:], in_=ps[0:KCH, :],
                                         func=IDENT, bias=bias[0:KCH, :],
                                         scale=1.0)
                else:
                    nc.vector.tensor_scalar_add(out=dst[:], in0=ps[0:KCH, :],
                                                scalar1=bias[0:KCH, :])

            fg_block(0, "f")
            g_emitted = {0, 1}
            for j in range(2):
                fg_block(j, "g")

            d_t = [None] * NSTRIP
            scr_t = [None] * NSTRIP
            COPY = mybir.ActivationFunctionType.Copy

            def hts_stage(s):
                """h matmul + hts scale for strip s (emitted 2 strips late so
                the GPS rowsum latency never blocks the DVE exp stream).  For
                late strips the scale runs on ACT, which idles during W1."""
                p, dlt = s // 2, s % 2
                scr, hw_ = scr_t[s]
                dd = small.tile([P, 1], F32, tag="dd", name=f"dd{s}")
                if True:
                    t2 = small.tile([P, 512], F16, tag="t2", name=f"t2_{s}")
                    nc.vector.tensor_add(out=t2[:, 0:hw_ // 2],
                                         in0=scr[:, 0:hw_ // 2],
                                         in1=scr[:, hw_ // 2:hw_])
                    t3 = small.tile([P, 256], F16, tag="t3", name=f"t3_{s}")
                    nc.vector.tensor_add(out=t3[:, 0:hw_ // 4],
                                         in0=t2[:, 0:hw_ // 4],
                                         in1=t2[:, hw_ // 4:hw_ // 2])
                    nc.vector.reduce_sum(out=dd[:], in_=t3[:, 0:hw_ // 4],
                                         axis=mybir.AxisListType.X)
                dt_ = small.tile([P, 1], F32, tag="dt", name=f"dt{s}")
                nc.vector.tensor_add(out=dt_[:], in0=d_t[s], in1=dd[:])
                rd = small.tile([P, 1], F32, tag="rd")
                nc.vector.reciprocal(out=rd[:], in_=dt_[:])
                ph = psA.tile([P, MBLK], F32, tag="att", name=f"ph{s}")
                nc.tensor.matmul(ph[:, 0:P],
                                 xbf_t[s // 4][:, (s % 4) * P:(s % 4 + 1) * P],
                                 wht_s, start=True, stop=False)
                nc.tensor.matmul(ph[:, 0:P], ones_s[:], bhr_s,
                                 start=False, stop=True)
                if HTS_ENG[s] == 'A':
                    rd2g = small.tile([P, 1], F32, tag="rd2g")
                    nc.vector.tensor_scalar(out=rd2g[:], in0=rd[:],
                                            scalar1=gam2k, op0=MUL,
                                            scalar2=None)
                    nc.scalar.activation(out=hts_p[p][:, dlt, :],
                                         in_=ph[:, 0:P], func=COPY,
                                         scale=rd2g[:])
                else:
                    nc.vector.tensor_scalar(out=hts_p[p][:, dlt, :],
                                            in0=ph[:, 0:P], scalar1=rd[:],
                                            scalar2=gam2k, op0=MUL, op1=MUL)

            next_hts = 0
            for s in range(NSTRIP):
                p, dlt = s // 2, s % 2
                if s % 4 == 1 and s // 4 + 1 < NMB:
                    fg_block(s // 4 + 1, "f")
                if s < 24:
                    while next_hts <= s - 2:
                        hts_stage(next_hts)
                        next_hts += 1

                accs = small.tile([P, 4], F32, tag="accs", name=f"accs{s}")
                fsl = f_sb[s // 4][0:KCH, (s % 4) * P:(s % 4 + 1) * P]
                for cix, (coff, cwd) in enumerate(CHUNKS):
                    if s <= 1 and cix >= 1:
                        for gb in range(2 * cix, 2 * cix + 2):
                            if gb not in g_emitted:
                                g_emitted.add(gb)
                                fg_block(gb, "g")
                    sps = psS.tile([P, 1024], F32, tag="sps",
                                   name=f"sps{s}_{cix}")
                    for half in range(cwd // MBLK):
                        off = half * MBLK
                        gb = (coff + off) // MBLK
                        # fp8 DoubleRow at 0.5 cyc/col: broadcast a fake
                        # Ko=2 dim (stride 0) on both operands; each product
                        # is summed twice, compensated by the 1/sqrt(2) in
                        # the f/g scales (kappa: g-side bias 0.5).
                        nc.tensor.matmul(
                            sps[:, off:off + MBLK],
                            fsl.unsqueeze(1).broadcast_to([KCH, 2, P]),
                            g_sb[gb][0:KCH, :].unsqueeze(1)
                            .broadcast_to([KCH, 2, MBLK]),
                            start=True, stop=True, perf_mode=DRMODE)
                    eout = e_p[p][:, dlt, coff:coff + cwd]
                    if CHUNK_ENG[s][cix] == 'A':
                        nc.scalar.activation(
                            out=eout, in_=sps[:, 0:cwd], func=EXP,
                            bias=ebias, scale=1.0 / S1,
                            accum_out=accs[:, ACC_SLOT[s][cix]:
                                           ACC_SLOT[s][cix] + 1]
                            if CHUNK_ACC[s][cix] else None)
                    else:
                        nc.vector.tensor_scalar(
                            out=eout.bitcast(I8), in0=sps[:, 0:cwd],
                            scalar1=0.0, scalar2=CLAMP_HI, op0=MAX, op1=MIN)

                # GPS rowsum over the non-accum chunks; the DVE reduce
                # + combine are deferred into hts_stage (2 strips later) so
                # the DVE exp stream never waits on GPS.
                d = small.tile([P, 1], F32, tag="d", name=f"d{s}")
                nr = [cix for cix in range(4) if not CHUNK_ACC[s][cix]]
                hw_ = (len(nr) // 2) * 1024 if len(nr) > 1 else 512
                scr = scrp.tile([P, 1024], F16, tag="scr", name=f"scr{s}")
                scr_t[s] = (scr, hw_)
                if len(nr) == 2:
                    i0 = e_p[p][:, dlt, nr[0] * 1024:(nr[0] + 1) * 1024]
                    i1 = e_p[p][:, dlt, nr[1] * 1024:(nr[1] + 1) * 1024]
                else:
                    i0 = e_p[p][:, dlt, nr[0] * 1024:nr[0] * 1024 + 512]
                    i1 = e_p[p][:, dlt, nr[0] * 1024 + 512:(nr[0] + 1) * 1024]
                nc.gpsimd.tensor_add(out=scr[:, 0:hw_], in0=i0, in1=i1)
                if sum(CHUNK_ACC[s]) == 3:
                    d1 = small.tile([P, 1], F32, tag="d1", name=f"d1_{s}")
                    nc.gpsimd.tensor_add(out=d1[:], in0=accs[:, 0:1],
                                         in1=accs[:, 1:2])
                    nc.gpsimd.tensor_add(out=d[:], in0=d1[:],
                                         in1=accs[:, 2:3])
                else:
                    nc.gpsimd.tensor_add(out=d[:], in0=accs[:, 0:1],
                                         in1=accs[:, 1:2])
                d_t[s] = d[:]

                if s >= 24:
                    while next_hts <= s - 1:
                        hts_stage(next_hts)
                        next_hts += 1
                    j = s - 24
                    pa = psA.tile([P, MBLK], F32, tag="att", name=f"pa{j}")
                    nw0 = W0J[j]
                    for q in range(nw0):
                        nc.tensor.matmul(
                            pa[:], hts_p[q][:, 0:2, :],
                            e_p[q][:, 0:2, j * MBLK:(j + 1) * MBLK],
                            start=(q == 0), stop=(q == nw0 - 1),
                            perf_mode=DRMODE)
                    nc.vector.scalar_tensor_tensor(
                        out=att_t[j][:], in0=pa[:], scalar=inv2k,
                        in1=x32_t[j][:].bitcast(F32), op0=MUL, op1=ADD)

            hts_stage(30)
            hts_stage(31)

            OUT_Q = [getattr(nc, e).dma_start for e in OUT_DMA_ENGS]
            # tail: window-1 chains (remaining pairs), fold, store.
            # FOLD_W1 'A' blocks fold on ACT into att2 then GPS-add (both
            # idle in the tail); 'V' blocks use the one-op DVE stt.
            for j in range(NMB):
                pa = psA.tile([P, MBLK], F32, tag="att", name=f"tpa{j}")
                for q in range(W0J[j], NPAIR):
                    nc.tensor.matmul(
                        pa[:], hts_p[q][:, 0:2, :],
                        e_p[q][:, 0:2, j * MBLK:(j + 1) * MBLK],
                        start=(q == W0J[j]), stop=(q == NPAIR - 1),
                        perf_mode=DRMODE)
                if FOLD_W1[j] == 'A':
                    att2 = consts.tile([P, MBLK], F32,
                                       tag=f"att2_{j % 2}",
                                       name=f"att2_{j}")
                    nc.scalar.activation(out=att2[:], in_=pa[:],
                                         func=COPY, scale=inv2k)
                    nc.gpsimd.tensor_add(out=att_t[j][:], in0=att_t[j][:],
                                         in1=att2[:])
                else:
                    nc.vector.scalar_tensor_tensor(
                        out=att_t[j][:], in0=pa[:], scalar=inv2k,
                        in1=att_t[j][:], op0=MUL, op1=ADD)
                OUT_Q[j % len(OUT_Q)](out[:, j * MBLK:(j + 1) * MBLK],
                                       att_t[j][:])

    nc.compile()
    return nc


def _get_nc():
    global _NC
    if _NC is None:
        _NC = _build()
    return _NC


def _prep_weights(Wf, bf, Wg, bg, Wh, bh, gamma):
    bf16 = ml_dtypes.bfloat16
    # S matmul runs DoubleRow with a broadcast Ko dim: every f.g product is
    # summed twice, so fold 1/sqrt(2) into each of f and g.
    sq = SQS1 / math.sqrt(2.0)
    wpack = np.zeros((P, 320), np.float32)
    wpack[:, 0:16] = Wf.T * sq
    wpack[:, 32:48] = Wg.T * sq
    wpack[:, 64:192] = Wh.T
    wpack[0, 192:320] = bh
    g = float(np.float32(np.asarray(gamma).reshape(())))
    # 2^KP scales hts into fp8e4 range: hts ~ h * gamma * 2^K / D with
    # D ~ 4096*exp(-2)*E[e^S] ~ 1250 and |h| ~ 0.57.
    KP = int(np.clip(np.round(np.log2(2200.0 / max(abs(g), 1e-8))), 0, 60))
    fpack = np.zeros((P, 5), np.float32)
    fpack[0:16, 0] = bf * sq
    fpack[16, 0] = CK1
    fpack[17, 0] = CK2
    # kappa g-bias 0.5: doubled kappa product restores CK1+CK2 exactly
    fpack[16, 1] = 0.5
    fpack[17, 1] = 0.5
    fpack[:, 2] = g * (2.0 ** KP)
    fpack[:, 3] = EXP_BIAS
    fpack[:, 4] = 2.0 ** (-KP)
    return {"wpack": wpack.astype(bf16), "fpack": fpack}


def make_in_maps(x, Wf, bf, Wg, bg, Wh, bh, gamma):
    bf16 = ml_dtypes.bfloat16
    wmap = _prep_weights(np.asarray(Wf), np.asarray(bf), np.asarray(Wg),
                         np.asarray(bg), np.asarray(Wh), np.asarray(bh),
                         np.asarray(gamma))
    xf = np.ascontiguousarray(np.asarray(x, np.float32).reshape(B, C, N))
    in_maps = []
    for b in range(B):
        m = dict(wmap)
        m["x32"] = xf[b]
        m["xbf"] = xf[b].astype(bf16)
        in_maps.append(m)
    return in_maps


def kernel(x, Wf, bf, Wg, bg, Wh, bh, gamma):
    nc = _get_nc()
    in_maps = make_in_maps(x, Wf, bf, Wg, bg, Wh, bh, gamma)
    res = run_bass_kernel_spmd(nc, in_maps, core_ids=list(range(B)))
    out = np.stack([res.results[b]["out"] for b in range(B)], axis=0)
    return out.reshape(B, C, W, H).astype(np.float32)

